# revision 40
# baseline (speedup 1.0000x reference)
"""KNN retrieval kernel for Trainium2 (8 NeuronCores, data-parallel over queries).

Problem: for each query row x[i] (N=16384, DIM=16), find j* = argmin_j ||xb[j]-x[i]||
over M=16384 reference rows and return y[j*].

Device algorithm (per core, 2048 queries):
  ms[i,j] = 2<x_i, xb_j> - ||xb_j||^2   (argmax_j ms == argmin_j dist; the
            ||x_i||^2 term is constant per row and dropped)
  - PE: ms computed as K=17 matmuls (16 dims + 1 augmented row carrying
    -||xb_j||^2), 4 j-tiles packed into the 128x128 array via 32-row groups.
  - DVE: chained tensor_tensor_scan(max) turns each 16384-wide row of ms
    (read straight from PSUM) into its running prefix-max, written to SBUF.
    The last column is the row max g.
  - ACT: one Sign activation with accum computes
        j* = sum_j sign(g - prefix[j]) = #{j : prefix[j] < g}
    which is exactly the first-occurrence argmax index (ties included).
  - GPSIMD: indirect DMA gathers y[j*] from DRAM.
Host: builds augmented/packed layouts, shards queries 8 ways, reassembles.
"""

import os
import sys

sys.path.insert(0, "/opt/trn_rl_repo")

import numpy as np

N, M, DIM = 16384, 16384, 16
NCORES = 8
NQ = N // NCORES  # queries per core
RB = 128          # row-block (queries per partition block)
JT = 512          # j-tile width (one PSUM bank of fp32)
TPG = 4           # j-tiles packed per PE group (32-row groups)
CHUNK = TPG * JT  # scan chunk width (4 PSUM banks)
K_AUG = 17        # 16 dims + 1 augmentation row
K_SPL = 50        # bf16-split contraction: 16 hi + 2 aug + 16 lo + 16 hi


WSUB = 32         # sub-block width for the submax algorithm
FILLER_LDW = 8    # no-op ldweights after each chunk to hold PE p-state


def build_nc(nq=NQ, m=M, mode="fp32", loop_n=0, parts="full",
             count_engine="act", algo="scan"):
    """Build the per-core Bass module. loop_n>0 wraps the compute in a
    hardware repeat loop (for timing measurement only). parts in
    {"full", "mm", "mmscan"} selects pipeline stages (for perf bisection)."""
    import contextlib
    from contextlib import ExitStack

    import concourse.bacc as bacc
    import concourse.bass as bass
    import concourse.mybir as mybir
    import concourse.tile as tile
    from concourse.bass import IndirectOffsetOnAxis

    fp32 = mybir.dt.float32
    fp16 = mybir.dt.float16
    n_rb = nq // RB
    n_chunk = m // CHUNK
    NEGINF = float(np.float32(-3.0e38))

    nc = bacc.Bacc("TRN2", target_bir_lowering=False, debug=False)

    if mode == "fp8dr":
        # fp8 e4m3 hi/lo split, DoubleRow: operands [34, 2, cols], K_eff=68
        fp8 = mybir.dt.float8e4
        xq_d = nc.dram_tensor("xq8", [34, 2 * nq], fp8, kind="ExternalInput")
        xb_d = nc.dram_tensor("xb8", [34, 2 * m], fp8, kind="ExternalInput")
    else:
        in_dt = mybir.dt.bfloat16 if mode == "bf16split" else fp32
        xb_free = n_chunk * (2 if mode == "bf16split" else TPG) * JT
        xq_d = nc.dram_tensor("xq4", [128, nq], in_dt, kind="ExternalInput")
        xb_d = nc.dram_tensor("xbp", [128, xb_free], in_dt,
                              kind="ExternalInput")
    y_d = nc.dram_tensor("ytab", [m, 1], fp32, kind="ExternalInput")
    out_d = nc.dram_tensor("yout", [128, n_rb], fp32, kind="ExternalOutput")
    if algo in ("submax", "submax16", "fold16"):
        xw_d = nc.dram_tensor("xw", [m // WSUB, K_AUG * WSUB], fp32,
                              kind="ExternalInput")
        xqr_d = nc.dram_tensor("xqr", [128, n_rb * K_AUG], fp32,
                               kind="ExternalInput")

    with tile.TileContext(nc) as tc:
        with ExitStack() as ctx:
            consts = ctx.enter_context(tc.tile_pool(name="consts", bufs=1))
            psum_pool = ctx.enter_context(
                tc.tile_pool(name="ps", bufs=2, space=bass.MemorySpace.PSUM))
            pms_pool = ctx.enter_context(tc.tile_pool(name="pms", bufs=3))
            gpool = ctx.enter_context(tc.tile_pool(name="g", bufs=2))
            outp = ctx.enter_context(tc.tile_pool(name="outp", bufs=1))

            assert n_chunk % 2 == 0
            half_chunks = n_chunk // 2
            half = half_chunks * CHUNK

            if mode == "fp8dr":
                fp8 = mybir.dt.float8e4
                xq4 = consts.tile([34, 2 * nq], fp8)
                xb = consts.tile([34, 2 * m], fp8)
                nc.sync.dma_start(xq4[:], xq_d[:])
                nc.sync.dma_start(xb[:], xb_d[:])
                xq8v = xq4[:].rearrange("p (two c) -> p two c", two=2)
                xb8v = xb[:].rearrange("p (two c) -> p two c", two=2)
            else:
                xq4 = consts.tile([128, nq], in_dt)
                xb = consts.tile([128, xb_free], in_dt)
                nc.sync.dma_start(xq4[:], xq_d[:])
                nc.sync.dma_start(xb[:], xb_d[:])
            if mode == "bf16split":
                dummy = consts.tile([128, CHUNK], fp32)
                nc.vector.memset(dummy[:], 0.0)

            J0 = outp.tile([128, n_rb], fp32)
            J1 = outp.tile([128, n_rb], fp32)
            Yg = outp.tile([128, n_rb], fp32)
            if parts != "full":
                nc.gpsimd.memset(Yg[:], 0.0)

            def emit_mms(rb, t, ps):
                if mode == "fp8dr":
                    for u in range(TPG):
                        nc.tensor.matmul(
                            ps[:, u * JT:(u + 1) * JT],
                            xq8v[:, :, rb * RB:(rb + 1) * RB],
                            xb8v[:, :, (t * TPG + u) * JT:
                                 (t * TPG + u + 1) * JT],
                            start=True,
                            stop=True,
                            perf_mode=mybir.MatmulPerfMode.DoubleRow,
                        )
                elif mode == "bf16split":
                    # K=50 split-bf16 contraction, 2-way row packing
                    for v in range(2):
                        for s in range(2):
                            u = 2 * v + s
                            nc.tensor.matmul(
                                ps[:, u * JT:(u + 1) * JT],
                                xq4[64 * s:64 * s + K_SPL,
                                    rb * RB:(rb + 1) * RB],
                                xb[64 * s:64 * s + K_SPL,
                                   (t * 2 + v) * JT:(t * 2 + v + 1) * JT],
                                start=True,
                                stop=True,
                                tile_position=(64 * s, 0),
                            )
                    # keep PE busy through the tile-wait gap so the p-state
                    # ramp (full speed needs ~3us continuous) is not reset
                    for _ in range(FILLER_LDW):
                        nc.tensor.ldweights(
                            xq4[0:K_SPL, rb * RB:(rb + 1) * RB],
                            tile_position=(0, 0),
                        )
                else:
                    for b in range(TPG):
                        nc.tensor.matmul(
                            ps[:, b * JT:(b + 1) * JT],
                            xq4[32 * b:32 * b + K_AUG,
                                rb * RB:(rb + 1) * RB],
                            xb[32 * b:32 * b + K_AUG,
                               (t * TPG + b) * JT:(t * TPG + b + 1) * JT],
                            start=True,
                            stop=True,
                            tile_position=(32 * b, 0),
                        )

            if algo in ("submax", "submax16", "fold16"):
                xqr = consts.tile([128, n_rb * K_AUG], fp32)
                nc.sync.dma_start(xqr[:], xqr_d[:])
                smpool = ctx.enter_context(tc.tile_pool(name="sm", bufs=2))
                wpool = ctx.enter_context(tc.tile_pool(name="w", bufs=2))
                nsub = m // WSUB
                cps = CHUNK // WSUB  # sub-blocks per chunk
                WK = WSUB * K_AUG
            if algo == "submax16":
                hpool = ctx.enter_context(tc.tile_pool(name="h16", bufs=3))
            if algo == "fold16":
                # fold drain needs all 8 chunk copies of an rb live, plus
                # headroom to overlap the next rb's copies.
                hpool = ctx.enter_context(tc.tile_pool(name="h16", bufs=10))
                fpool = ctx.enter_context(tc.tile_pool(name="f16", bufs=8))

            sm_dt = fp16 if algo in ("submax16", "fold16") else fp32

            def emit_fold16_rb(rb):
                """Cross-chunk fold drain. Host packs PSUM column (t, u, w)
                with augmented column 32*w + 4*t + u, so pairwise tt-max
                folds over chunks, then over halves, land exactly on the
                32-wide sub-blocks: SM[v] = max of original block v."""
                assert n_chunk == 8 and CHUNK == 2048 and nsub == 512
                SM = smpool.tile([128, nsub], fp16)
                hs = []
                for t in range(n_chunk):
                    ps = psum_pool.tile([128, CHUNK], fp32, name=f"p{rb}_{t}",
                                        tag="ps")
                    emit_mms(rb, t, ps)
                    h = hpool.tile([128, CHUNK], fp16, name=f"h{rb}_{t}",
                                   tag="h")
                    nc.scalar.copy(h[:], ps[:])
                    hs.append(h)
                    if parts == "mm":
                        nc.vector.tensor_copy(SM[:, t * 8:(t + 1) * 8],
                                              ps[:, 0:8])
                MAX = mybir.AluOpType.max
                if parts == "mm":
                    return
                lvl = hs
                li = 0
                while len(lvl) > 1:
                    nxt = []
                    for a in range(0, len(lvl), 2):
                        f = fpool.tile([128, CHUNK], fp16,
                                       name=f"f{rb}_{li}_{a}", tag="f")
                        nc.vector.tensor_tensor(f[:], lvl[a][:],
                                                lvl[a + 1][:], op=MAX)
                        nxt.append(f)
                    lvl = nxt
                    li += 1
                A = lvl[0]
                B = fpool.tile([128, 1024], fp16, name=f"fb{rb}", tag="fb")
                nc.vector.tensor_tensor(B[:], A[:, 0:1024], A[:, 1024:2048],
                                        op=MAX)
                nc.vector.tensor_tensor(SM[:], B[:, 0:512], B[:, 512:1024],
                                        op=MAX)
                if parts != "full":
                    return None
                return emit_select_rb(rb, SM)

            def emit_submax_rb(rb):
                SM = smpool.tile([128, nsub], sm_dt)
                for t in range(n_chunk):
                    ps = psum_pool.tile([128, CHUNK], fp32, name=f"p{rb}_{t}",
                                        tag="ps")
                    emit_mms(rb, t, ps)
                    if parts == "mm":
                        nc.vector.tensor_copy(SM[:, t * cps:t * cps + 8],
                                              ps[:, 0:8])
                        continue
                    if algo == "submax16":
                        # ACT drains PSUM to fp16 SBUF; DVE then block-max
                        # reduces the fp16 copy in 2x perf mode.
                        h = hpool.tile([128, CHUNK], fp16, name=f"h{rb}_{t}",
                                       tag="h")
                        nc.scalar.copy(h[:], ps[:])
                        red_src = h[:].rearrange("p (s w) -> p s w", w=WSUB)
                    else:
                        red_src = ps[:].rearrange("p (s w) -> p s w", w=WSUB)
                    nc.vector.tensor_reduce(
                        SM[:, t * cps:(t + 1) * cps],
                        red_src,
                        mybir.AxisListType.X,
                        mybir.AluOpType.max,
                    )
                if parts != "full":
                    return None
                return emit_select_rb(rb, SM)

            def emit_select_rb(rb, SM):
                """Top-2 sub-block selection + window gather LAUNCH. The
                re-dot (emit_redot_rb) is deferred one rb so the indirect
                DMA latency hides under the next rb's drain."""
                m8 = gpool.tile([128, 8], sm_dt)
                i8 = gpool.tile([128, 8], mybir.dt.uint32)
                nc.vector.max(m8[:], SM[:])
                nc.vector.max_index(i8[:], m8[:], SM[:])
                slo = gpool.tile([128, 1], mybir.dt.uint32)
                shi = gpool.tile([128, 1], mybir.dt.uint32)
                nc.vector.tensor_tensor(slo[:], i8[:, 0:1], i8[:, 1:2],
                                        op=mybir.AluOpType.min)
                nc.vector.tensor_tensor(shi[:], i8[:, 0:1], i8[:, 1:2],
                                        op=mybir.AluOpType.max)
                Wlo = wpool.tile([128, WK], fp32)
                Whi = wpool.tile([128, WK], fp32)
                nc.gpsimd.indirect_dma_start(
                    Wlo[:], None, xw_d[:], IndirectOffsetOnAxis(slo[:], 0))
                nc.gpsimd.indirect_dma_start(
                    Whi[:], None, xw_d[:], IndirectOffsetOnAxis(shi[:], 0))
                return dict(slo=slo, shi=shi, Wlo=Wlo, Whi=Whi)

            def emit_redot_rb(rb, st):
                slo, shi, Wlo, Whi = st["slo"], st["shi"], st["Wlo"], st["Whi"]
                # exact fp32 re-dot of the two candidate windows (window
                # rows are c-major so inner k is packed)
                xq_b = (xqr[:, rb * K_AUG:(rb + 1) * K_AUG]
                        .rearrange("p (c k) -> p c k", c=1)
                        .to_broadcast([128, WSUB, K_AUG]))
                Dt = wpool.tile([128, 2 * WK], fp32)
                Dd = wpool.tile([128, 2 * WSUB], fp32)
                for wi, Wt in ((0, Wlo), (1, Whi)):
                    dt_v = Dt[:, wi * WK:(wi + 1) * WK].rearrange(
                        "p (c k) -> p c k", k=K_AUG)
                    nc.vector.tensor_tensor(
                        dt_v, Wt[:].rearrange("p (c k) -> p c k", k=K_AUG),
                        xq_b, op=mybir.AluOpType.mult)
                    nc.vector.tensor_reduce(
                        Dd[:, wi * WSUB:(wi + 1) * WSUB], dt_v,
                        mybir.AxisListType.X, mybir.AluOpType.add)
                cm8 = gpool.tile([128, 8], fp32)
                ci8 = gpool.tile([128, 8], mybir.dt.uint32)
                nc.vector.max(cm8[:], Dd[:])
                nc.vector.max_index(ci8[:], cm8[:], Dd[:])
                # j* = (c2<W ? slo : shi)*W + c2 mod W, all in fp32
                c2f = gpool.tile([128, 1], fp32)
                slof = gpool.tile([128, 1], fp32)
                shif = gpool.tile([128, 1], fp32)
                ge = gpool.tile([128, 1], fp32)
                t1 = gpool.tile([128, 1], fp32)
                jf = gpool.tile([128, 1], fp32)
                nc.vector.tensor_copy(c2f[:], ci8[:, 0:1])
                nc.vector.tensor_copy(slof[:], slo[:])
                nc.vector.tensor_copy(shif[:], shi[:])
                nc.vector.tensor_scalar(
                    out=ge[:], in0=c2f[:], scalar1=float(WSUB), scalar2=None,
                    op0=mybir.AluOpType.is_ge)
                nc.vector.tensor_sub(t1[:], shif[:], slof[:])
                nc.vector.tensor_mul(t1[:], ge[:], t1[:])
                nc.vector.tensor_add(t1[:], slof[:], t1[:])  # chosen s
                nc.vector.scalar_tensor_tensor(
                    jf[:], t1[:], float(WSUB), c2f[:],
                    mybir.AluOpType.mult, mybir.AluOpType.add)
                nc.vector.scalar_tensor_tensor(
                    jf[:], ge[:], float(-WSUB), jf[:],
                    mybir.AluOpType.mult, mybir.AluOpType.add)
                ji = gpool.tile([128, 1], mybir.dt.uint32)
                nc.vector.tensor_copy(ji[:], jf[:])
                nc.gpsimd.indirect_dma_start(
                    Yg[:, rb:rb + 1], None, y_d[:],
                    IndirectOffsetOnAxis(ap=ji[:], axis=0))

            loop_cm = (tc.For_i(0, loop_n, 1) if loop_n
                       else contextlib.nullcontext())
            with loop_cm:
              pend = None  # (rb, select-state) awaiting its re-dot
              for rb in range(n_rb):
                if algo == "fold16":
                    st = emit_fold16_rb(rb)
                    if pend is not None:
                        emit_redot_rb(*pend)
                    pend = (rb, st) if st is not None else None
                    continue
                if algo in ("submax", "submax16"):
                    st = emit_submax_rb(rb)
                    if st is not None:
                        emit_redot_rb(rb, st)
                    continue
                # prefix-max of the row is built in two half-row tiles
                halves = [pms_pool.tile([128, half], fp32, name=f"pm{rb}_{h}",
                                        tag="pmh")
                          for h in range(2)]
                for t in range(n_chunk):
                    ps = psum_pool.tile([128, CHUNK], fp32)
                    emit_mms(rb, t, ps)
                    h, tc_ = divmod(t, half_chunks)
                    if parts == "mm":
                        # consume a sliver of PSUM so matmuls are not dead
                        nc.vector.tensor_copy(
                            halves[h][:, tc_ * CHUNK:tc_ * CHUNK + 8],
                            ps[:, 0:8])
                        continue
                    if t == 0:
                        initial = NEGINF
                    elif tc_ == 0:
                        initial = halves[h - 1][:, half - 1:half]
                    else:
                        initial = halves[h][:, tc_ * CHUNK - 1:tc_ * CHUNK]
                    # prefix-max of this chunk, chained to the previous chunk;
                    # data1 is an ignored operand (op1=bypass) shaped like data0.
                    nc.vector.tensor_tensor_scan(
                        halves[h][:, tc_ * CHUNK:(tc_ + 1) * CHUNK],
                        ps[:],
                        dummy[:] if mode == "bf16split" else xb[:, 0:CHUNK],
                        initial,
                        mybir.AluOpType.max,
                        mybir.AluOpType.bypass,
                    )
                if parts != "full":
                    continue
                gt = gpool.tile([128, 1], fp32)
                nc.vector.tensor_copy(gt[:], halves[1][:, half - 1:half])
                # j* = sum_j sign(g - prefix[j]) = #{j: prefix[j] < g};
                # in-place output over the prefix tiles, one accumulator per
                # half, summed later. count_engine picks ACT sign-accum or
                # DVE is_lt-accum (2x mode) per half.
                for h, Jh in ((0, J0), (1, J1)):
                    eng = {"act": "act", "dve": "dve",
                           "split": "act" if h == 0 else "dve"}[count_engine]
                    if eng == "act":
                        nc.scalar.activation(
                            halves[h][:, :],
                            halves[h][:, :],
                            mybir.ActivationFunctionType.Sign,
                            bias=gt[:],
                            scale=-1.0,
                            accum_out=Jh[:, rb:rb + 1],
                        )
                    else:
                        nc.vector.tensor_scalar(
                            out=halves[h][:, :],
                            in0=halves[h][:, :],
                            scalar1=gt[:],
                            scalar2=None,
                            op0=mybir.AluOpType.is_lt,
                            op1=mybir.AluOpType.add,
                            accum_out=Jh[:, rb:rb + 1],
                        )
                # j* for this row-block -> uint32 -> gather y[j*] from DRAM
                ji = gpool.tile([128, 1], mybir.dt.uint32, name=f"ji{rb}",
                                tag="ji")
                nc.vector.scalar_tensor_tensor(
                    ji[:], J0[:, rb:rb + 1], 1.0, J1[:, rb:rb + 1],
                    mybir.AluOpType.mult, mybir.AluOpType.add,
                )
                nc.gpsimd.indirect_dma_start(
                    Yg[:, rb:rb + 1],
                    None,
                    y_d[:],
                    IndirectOffsetOnAxis(ap=ji[:], axis=0),
                )

              if pend is not None:
                  emit_redot_rb(*pend)

            nc.sync.dma_start(out_d[:], Yg[:])

    nc.compile()
    return nc


def prep_inputs(x, xb, y, nq=NQ, m=M, mode="fp32", algo="scan"):
    """Host-side packing. Returns per-core input maps (shared arrays reused)."""
    x = np.asarray(x, dtype=np.float32)
    xb = np.asarray(xb, dtype=np.float32)
    y = np.asarray(y, dtype=np.float32)
    n_chunk = m // CHUNK
    n_rb = nq // RB
    ncores = x.shape[0] // nq
    ytab = np.ascontiguousarray(y.reshape(m, 1))
    in_maps = []

    extra = {}
    if algo in ("submax", "submax16", "fold16"):
        xaug = np.empty((K_AUG, m), np.float32)
        xaug[:DIM] = 2.0 * xb.T
        xaug[DIM] = -np.einsum("ij,ij->i", xb, xb)
        # window rows are c-major ([WSUB, K_AUG]) so the re-dot views are
        # packed (inner k stride 1) for both DVE and GPSIMD
        extra["xw"] = np.ascontiguousarray(
            xaug.reshape(K_AUG, m // WSUB, WSUB).transpose(1, 2, 0)
            .reshape(m // WSUB, K_AUG * WSUB))

    # fold16: PSUM column (chunk t, quadrant u, col w) holds augmented
    # column 32*w + 4*t + u, so the device's cross-chunk + half folds land
    # on the 32-wide sub-blocks (SM[v] = max of original block v).
    if algo == "fold16":
        tt = np.arange(n_chunk)[:, None, None]
        uu = np.arange(TPG)[None, :, None]
        ww = np.arange(JT)[None, None, :]
        col_map = (WSUB * ww + TPG * tt + uu).reshape(-1)  # [(t u w)]
    else:
        col_map = np.arange(m)

    def add_core_extras(core_maps, c):
        if algo not in ("submax", "submax16", "fold16"):
            return
        arr = np.ones((128, n_rb, K_AUG), np.float32)
        arr[:, :, :DIM] = x[c * nq:(c + 1) * nq].reshape(
            n_rb, RB, DIM).transpose(1, 0, 2)
        core_maps["xqr"] = np.ascontiguousarray(arr.reshape(128, -1))
        core_maps["xw"] = extra["xw"]

    if mode == "fp8dr":
        import ml_dtypes

        e4 = ml_dtypes.float8_e4m3fn

        def sp8(a):
            hi = a.astype(e4).astype(np.float32)
            return hi, (a - hi).astype(e4).astype(np.float32)

        A = 2.0 * xb.T                       # [16, m]
        ah, al = sp8(A)
        b2 = -np.einsum("ij,ij->i", xb, xb)  # [m]
        b2h, b2l = sp8(b2)
        # 34 rows x 2 subtiles; subtile planes share the same ref rows:
        # rows 0..15: ah, 16: b2h, 17..32: al, 33: b2l
        RB8 = np.empty((34, m), np.float32)
        RB8[0:16] = ah
        RB8[16] = b2h
        RB8[17:33] = al
        RB8[33] = b2l
        RB8 = RB8[:, col_map]                # fold16 column interleave
        xb8 = np.empty((34, 2, m), np.float32)
        xb8[:, 0] = RB8
        xb8[:, 1] = RB8
        xb8 = np.ascontiguousarray(xb8.reshape(34, 2 * m)).astype(e4)

        for c in range(ncores):
            xq = x[c * nq:(c + 1) * nq].T    # [16, nq]
            xh, xl = sp8(xq)
            Q = np.zeros((34, 2, nq), np.float32)
            Q[0:16, 0] = xh
            Q[16, 0] = 1.0
            Q[17:33, 0] = xh
            Q[33, 0] = 1.0
            Q[0:16, 1] = xl
            Q[17:33, 1] = xl
            im = {"xq8": np.ascontiguousarray(Q.reshape(34, 2 * nq)).astype(e4),
                  "xb8": xb8, "ytab": ytab}
            add_core_extras(im, c)
            in_maps.append(im)
        return in_maps

    if mode == "bf16split":
        import ml_dtypes

        bf16 = ml_dtypes.bfloat16

        def bf(a):
            return a.astype(bf16).astype(np.float32)

        a = 2.0 * xb.T                      # [16, m]
        ah, al = bf(a), a - bf(a)
        b2 = -np.einsum("ij,ij->i", xb, xb)  # [m]
        b2h, b2l = bf(b2), b2 - bf(b2)
        R = np.zeros((K_SPL, m), np.float32)
        R[0:16] = ah
        R[16] = b2h
        R[17] = b2l
        R[18:34] = ah
        R[34:50] = al
        Rr = R[:, col_map].reshape(K_SPL, n_chunk, TPG, JT)  # u = 2*v + s on axis 2
        XB2 = np.zeros((128, n_chunk * 2, JT), np.float32)
        # strip s handles u in {s, 2+s}; its column block (t*2+v) holds u=2v+s
        for s in range(2):
            XB2[64 * s:64 * s + K_SPL] = Rr[:, :, [s, 2 + s], :].transpose(
                0, 1, 2, 3).reshape(K_SPL, n_chunk * 2, JT)
        xbp = np.ascontiguousarray(
            XB2.reshape(128, n_chunk * 2 * JT)).astype(bf16)

        for c in range(ncores):
            xq = x[c * nq:(c + 1) * nq].T  # [16, nq]
            L = np.zeros((K_SPL, nq), np.float32)
            L[0:16] = bf(xq)
            L[16] = 1.0
            L[17] = 1.0
            L[18:34] = xq - bf(xq)
            L[34:50] = bf(xq)
            XQ2 = np.zeros((128, nq), np.float32)
            for s in range(2):
                XQ2[64 * s:64 * s + K_SPL] = L
            im = {"xq4": XQ2.astype(bf16), "xbp": xbp, "ytab": ytab}
            add_core_extras(im, c)
            in_maps.append(im)
        return in_maps

    # Augmented xb operand: rows 0..15 = 2*xb^T, row 16 = -||xb_j||^2.
    xaug = np.empty((K_AUG, m), np.float32)
    xaug[:DIM] = 2.0 * xb.T
    xaug[DIM] = -np.einsum("ij,ij->i", xb, xb)

    # xbp[32b+k, t*TPG+b, :] = xaug[k, col_map[t*CHUNK + b*JT : ... + JT]]
    xa = xaug[:, col_map].reshape(K_AUG, n_chunk, TPG, JT)
    xbp = np.zeros((128, n_chunk * TPG, JT), np.float32)
    for b in range(TPG):
        xbp[32 * b:32 * b + K_AUG, b::TPG, :] = xa[:, :, b, :]
    xbp = np.ascontiguousarray(xbp.reshape(128, n_chunk * TPG * JT))

    for c in range(ncores):
        xq = x[c * nq:(c + 1) * nq]  # [nq, 16]
        xq4 = np.zeros((128, nq), np.float32)
        for b in range(TPG):
            xq4[32 * b:32 * b + DIM] = xq.T
            xq4[32 * b + DIM] = 1.0
        im = {"xq4": xq4, "xbp": xbp, "ytab": ytab}
        add_core_extras(im, c)
        in_maps.append(im)
    return in_maps


def unpack_output(out_np, nq=NQ):
    """[128, n_rb] device layout -> [nq] query order."""
    return np.ascontiguousarray(out_np.T).reshape(nq)


_NC_CACHE = {}
MODE = "bf16split"
ALGO = "fold16"


def kernel(x, xb, y):
    import concourse.bass_utils as bass_utils

    key = (MODE, ALGO)
    if key not in _NC_CACHE:
        _NC_CACHE[key] = build_nc(mode=MODE, algo=ALGO)
    nc = _NC_CACHE[key]

    in_maps = prep_inputs(x, xb, y, mode=MODE, algo=ALGO)
    res = bass_utils.run_bass_kernel_spmd(nc, in_maps, core_ids=list(range(NCORES)))
    outs = [unpack_output(r["yout"]) for r in res.results]
    return np.concatenate(outs).astype(np.float32)


if __name__ == "__main__":
    # smoke test with random data against numpy reference
    rng = np.random.default_rng(0)
    x = rng.standard_normal((N, DIM), dtype=np.float32)
    xb = rng.standard_normal((M, DIM), dtype=np.float32)
    y = rng.random(M, dtype=np.float32)
    got = kernel(x, xb, y)
    d2 = (np.sum(x * x, 1)[:, None] + np.sum(xb * xb, 1)[None, :]
          - 2.0 * x @ xb.T)
    want = y[np.argmin(d2, axis=1)]
    err = np.abs(got - want)
    print("mismatches:", int((err > 0).sum()), "/", N)



# revision 41
# speedup vs baseline: 1.1703x; 1.1703x over previous
"""KNN retrieval kernel for Trainium2 (8 NeuronCores, data-parallel over queries).

Problem: for each query row x[i] (N=16384, DIM=16), find j* = argmin_j ||xb[j]-x[i]||
over M=16384 reference rows and return y[j*].

Device algorithm (per core, 2048 queries):
  ms[i,j] = 2<x_i, xb_j> - ||xb_j||^2   (argmax_j ms == argmin_j dist; the
            ||x_i||^2 term is constant per row and dropped)
  - PE: ms computed as K=17 matmuls (16 dims + 1 augmented row carrying
    -||xb_j||^2), 4 j-tiles packed into the 128x128 array via 32-row groups.
  - DVE: chained tensor_tensor_scan(max) turns each 16384-wide row of ms
    (read straight from PSUM) into its running prefix-max, written to SBUF.
    The last column is the row max g.
  - ACT: one Sign activation with accum computes
        j* = sum_j sign(g - prefix[j]) = #{j : prefix[j] < g}
    which is exactly the first-occurrence argmax index (ties included).
  - GPSIMD: indirect DMA gathers y[j*] from DRAM.
Host: builds augmented/packed layouts, shards queries 8 ways, reassembles.
"""

import os
import sys

sys.path.insert(0, "/opt/trn_rl_repo")

import numpy as np

N, M, DIM = 16384, 16384, 16
NCORES = 8
NQ = N // NCORES  # queries per core
RB = 128          # row-block (queries per partition block)
JT = 512          # j-tile width (one PSUM bank of fp32)
TPG = 4           # j-tiles packed per PE group (32-row groups)
CHUNK = TPG * JT  # scan chunk width (4 PSUM banks)
K_AUG = 17        # 16 dims + 1 augmentation row
K_SPL = 50        # bf16-split contraction: 16 hi + 2 aug + 16 lo + 16 hi


WSUB = 32         # sub-block width for the submax algorithm
FILLER_LDW = 0    # no-op ldweights after each chunk to hold PE p-state


def build_nc(nq=NQ, m=M, mode="fp32", loop_n=0, parts="full",
             count_engine="act", algo="scan"):
    """Build the per-core Bass module. loop_n>0 wraps the compute in a
    hardware repeat loop (for timing measurement only). parts in
    {"full", "mm", "mmscan"} selects pipeline stages (for perf bisection)."""
    import contextlib
    from contextlib import ExitStack

    import concourse.bacc as bacc
    import concourse.bass as bass
    import concourse.mybir as mybir
    import concourse.tile as tile
    from concourse.bass import IndirectOffsetOnAxis

    fp32 = mybir.dt.float32
    fp16 = mybir.dt.float16
    n_rb = nq // RB
    n_chunk = m // CHUNK
    NEGINF = float(np.float32(-3.0e38))

    nc = bacc.Bacc("TRN2", target_bir_lowering=False, debug=False)

    if mode == "fp8dr":
        # fp8 e4m3 hi/lo split, DoubleRow: operands [34, 2, cols], K_eff=68
        fp8 = mybir.dt.float8e4
        xq_d = nc.dram_tensor("xq8", [34, 2 * nq], fp8, kind="ExternalInput")
        xb_d = nc.dram_tensor("xb8", [34, 2 * m], fp8, kind="ExternalInput")
    else:
        in_dt = mybir.dt.bfloat16 if mode == "bf16split" else fp32
        xb_free = n_chunk * (2 if mode == "bf16split" else TPG) * JT
        xq_d = nc.dram_tensor("xq4", [128, nq], in_dt, kind="ExternalInput")
        xb_d = nc.dram_tensor("xbp", [128, xb_free], in_dt,
                              kind="ExternalInput")
    y_d = nc.dram_tensor("ytab", [m, 1], fp32, kind="ExternalInput")
    out_d = nc.dram_tensor("yout", [128, n_rb], fp32, kind="ExternalOutput")
    if algo in ("submax", "submax16", "fold16"):
        xw_d = nc.dram_tensor("xw", [m // WSUB, K_AUG * WSUB], fp32,
                              kind="ExternalInput")
        xqr_d = nc.dram_tensor("xqr", [128, n_rb * K_AUG], fp32,
                               kind="ExternalInput")

    with tile.TileContext(nc) as tc:
        with ExitStack() as ctx:
            consts = ctx.enter_context(tc.tile_pool(name="consts", bufs=1))
            psum_pool = ctx.enter_context(
                tc.tile_pool(name="ps", bufs=2, space=bass.MemorySpace.PSUM))
            pms_pool = ctx.enter_context(tc.tile_pool(name="pms", bufs=3))
            gpool = ctx.enter_context(tc.tile_pool(name="g", bufs=2))
            outp = ctx.enter_context(tc.tile_pool(name="outp", bufs=1))

            assert n_chunk % 2 == 0
            half_chunks = n_chunk // 2
            half = half_chunks * CHUNK

            if mode == "fp8dr":
                fp8 = mybir.dt.float8e4
                xq4 = consts.tile([34, 2 * nq], fp8)
                xb = consts.tile([34, 2 * m], fp8)
                nc.sync.dma_start(xq4[:], xq_d[:])
                nc.sync.dma_start(xb[:], xb_d[:])
                xq8v = xq4[:].rearrange("p (two c) -> p two c", two=2)
                xb8v = xb[:].rearrange("p (two c) -> p two c", two=2)
            else:
                xq4 = consts.tile([128, nq], in_dt)
                xb = consts.tile([128, xb_free], in_dt)
                nc.sync.dma_start(xq4[:], xq_d[:])
                nc.sync.dma_start(xb[:], xb_d[:])
            if mode == "bf16split":
                dummy = consts.tile([128, CHUNK], fp32)
                nc.vector.memset(dummy[:], 0.0)

            J0 = outp.tile([128, n_rb], fp32)
            J1 = outp.tile([128, n_rb], fp32)
            Yg = outp.tile([128, n_rb], fp32)
            if parts != "full":
                nc.gpsimd.memset(Yg[:], 0.0)

            def emit_mms(rb, t, ps):
                if mode == "fp8dr":
                    for u in range(TPG):
                        nc.tensor.matmul(
                            ps[:, u * JT:(u + 1) * JT],
                            xq8v[:, :, rb * RB:(rb + 1) * RB],
                            xb8v[:, :, (t * TPG + u) * JT:
                                 (t * TPG + u + 1) * JT],
                            start=True,
                            stop=True,
                            perf_mode=mybir.MatmulPerfMode.DoubleRow,
                        )
                elif mode == "bf16split":
                    # K=50 split-bf16 contraction, 2-way row packing
                    for v in range(2):
                        for s in range(2):
                            u = 2 * v + s
                            nc.tensor.matmul(
                                ps[:, u * JT:(u + 1) * JT],
                                xq4[64 * s:64 * s + K_SPL,
                                    rb * RB:(rb + 1) * RB],
                                xb[64 * s:64 * s + K_SPL,
                                   (t * 2 + v) * JT:(t * 2 + v + 1) * JT],
                                start=True,
                                stop=True,
                                tile_position=(64 * s, 0),
                            )
                    # keep PE busy through the tile-wait gap so the p-state
                    # ramp (full speed needs ~3us continuous) is not reset
                    for _ in range(FILLER_LDW):
                        nc.tensor.ldweights(
                            xq4[0:K_SPL, rb * RB:(rb + 1) * RB],
                            tile_position=(0, 0),
                        )
                else:
                    for b in range(TPG):
                        nc.tensor.matmul(
                            ps[:, b * JT:(b + 1) * JT],
                            xq4[32 * b:32 * b + K_AUG,
                                rb * RB:(rb + 1) * RB],
                            xb[32 * b:32 * b + K_AUG,
                               (t * TPG + b) * JT:(t * TPG + b + 1) * JT],
                            start=True,
                            stop=True,
                            tile_position=(32 * b, 0),
                        )

            if algo in ("submax", "submax16", "fold16"):
                xqr = consts.tile([128, n_rb * K_AUG], fp32)
                nc.sync.dma_start(xqr[:], xqr_d[:])
                smpool = ctx.enter_context(tc.tile_pool(name="sm", bufs=2))
                wpool = ctx.enter_context(tc.tile_pool(name="w", bufs=2))
                nsub = m // WSUB
                cps = CHUNK // WSUB  # sub-blocks per chunk
                WK = WSUB * K_AUG
            if algo == "submax16":
                hpool = ctx.enter_context(tc.tile_pool(name="h16", bufs=3))
            if algo == "fold16":
                # fold drain needs all 8 chunk copies of an rb live, plus
                # headroom to overlap the next rb's copies.
                hpool = ctx.enter_context(tc.tile_pool(name="h16", bufs=10))
                fpool = ctx.enter_context(tc.tile_pool(name="f16", bufs=8))

            sm_dt = fp16 if algo in ("submax16", "fold16") else fp32

            def emit_fold16_rb(rb):
                """Cross-chunk fold drain. Host packs PSUM column (t, u, w)
                with augmented column 32*w + 4*t + u, so pairwise tt-max
                folds over chunks, then over halves, land exactly on the
                32-wide sub-blocks: SM[v] = max of original block v."""
                assert n_chunk == 8 and CHUNK == 2048 and nsub == 512
                SM = smpool.tile([128, nsub], fp16)
                hs = []
                for t in range(n_chunk):
                    ps = psum_pool.tile([128, CHUNK], fp32, name=f"p{rb}_{t}",
                                        tag="ps")
                    emit_mms(rb, t, ps)
                    h = hpool.tile([128, CHUNK], fp16, name=f"h{rb}_{t}",
                                   tag="h")
                    nc.scalar.copy(h[:], ps[:])
                    hs.append(h)
                    if parts == "mm":
                        nc.vector.tensor_copy(SM[:, t * 8:(t + 1) * 8],
                                              ps[:, 0:8])
                MAX = mybir.AluOpType.max
                if parts == "mm":
                    return
                lvl = hs
                li = 0
                while len(lvl) > 1:
                    nxt = []
                    for a in range(0, len(lvl), 2):
                        f = fpool.tile([128, CHUNK], fp16,
                                       name=f"f{rb}_{li}_{a}", tag="f")
                        nc.vector.tensor_tensor(f[:], lvl[a][:],
                                                lvl[a + 1][:], op=MAX)
                        nxt.append(f)
                    lvl = nxt
                    li += 1
                A = lvl[0]
                B = fpool.tile([128, 1024], fp16, name=f"fb{rb}", tag="fb")
                nc.vector.tensor_tensor(B[:], A[:, 0:1024], A[:, 1024:2048],
                                        op=MAX)
                nc.vector.tensor_tensor(SM[:], B[:, 0:512], B[:, 512:1024],
                                        op=MAX)
                if parts != "full":
                    return None
                return emit_select_rb(rb, SM)

            def emit_submax_rb(rb):
                SM = smpool.tile([128, nsub], sm_dt)
                for t in range(n_chunk):
                    ps = psum_pool.tile([128, CHUNK], fp32, name=f"p{rb}_{t}",
                                        tag="ps")
                    emit_mms(rb, t, ps)
                    if parts == "mm":
                        nc.vector.tensor_copy(SM[:, t * cps:t * cps + 8],
                                              ps[:, 0:8])
                        continue
                    if algo == "submax16":
                        # ACT drains PSUM to fp16 SBUF; DVE then block-max
                        # reduces the fp16 copy in 2x perf mode.
                        h = hpool.tile([128, CHUNK], fp16, name=f"h{rb}_{t}",
                                       tag="h")
                        nc.scalar.copy(h[:], ps[:])
                        red_src = h[:].rearrange("p (s w) -> p s w", w=WSUB)
                    else:
                        red_src = ps[:].rearrange("p (s w) -> p s w", w=WSUB)
                    nc.vector.tensor_reduce(
                        SM[:, t * cps:(t + 1) * cps],
                        red_src,
                        mybir.AxisListType.X,
                        mybir.AluOpType.max,
                    )
                if parts != "full":
                    return None
                return emit_select_rb(rb, SM)

            def emit_select_rb(rb, SM):
                """Top-2 sub-block selection + window gather LAUNCH. The
                re-dot (emit_redot_rb) is deferred one rb so the indirect
                DMA latency hides under the next rb's drain."""
                m8 = gpool.tile([128, 8], sm_dt)
                i8 = gpool.tile([128, 8], mybir.dt.uint32)
                nc.vector.max(m8[:], SM[:])
                nc.vector.max_index(i8[:], m8[:], SM[:])
                slo = gpool.tile([128, 1], mybir.dt.uint32)
                shi = gpool.tile([128, 1], mybir.dt.uint32)
                nc.vector.tensor_tensor(slo[:], i8[:, 0:1], i8[:, 1:2],
                                        op=mybir.AluOpType.min)
                nc.vector.tensor_tensor(shi[:], i8[:, 0:1], i8[:, 1:2],
                                        op=mybir.AluOpType.max)
                Wlo = wpool.tile([128, WK], fp32)
                Whi = wpool.tile([128, WK], fp32)
                nc.gpsimd.indirect_dma_start(
                    Wlo[:], None, xw_d[:], IndirectOffsetOnAxis(slo[:], 0))
                nc.gpsimd.indirect_dma_start(
                    Whi[:], None, xw_d[:], IndirectOffsetOnAxis(shi[:], 0))
                return dict(slo=slo, shi=shi, Wlo=Wlo, Whi=Whi)

            def emit_redot_rb(rb, st):
                slo, shi, Wlo, Whi = st["slo"], st["shi"], st["Wlo"], st["Whi"]
                # exact fp32 re-dot of the two candidate windows (window
                # rows are c-major so inner k is packed)
                xq_b = (xqr[:, rb * K_AUG:(rb + 1) * K_AUG]
                        .rearrange("p (c k) -> p c k", c=1)
                        .to_broadcast([128, WSUB, K_AUG]))
                Dt = wpool.tile([128, 2 * WK], fp32)
                Dd = wpool.tile([128, 2 * WSUB], fp32)
                for wi, Wt in ((0, Wlo), (1, Whi)):
                    dt_v = Dt[:, wi * WK:(wi + 1) * WK].rearrange(
                        "p (c k) -> p c k", k=K_AUG)
                    nc.vector.tensor_tensor(
                        dt_v, Wt[:].rearrange("p (c k) -> p c k", k=K_AUG),
                        xq_b, op=mybir.AluOpType.mult)
                    nc.vector.tensor_reduce(
                        Dd[:, wi * WSUB:(wi + 1) * WSUB], dt_v,
                        mybir.AxisListType.X, mybir.AluOpType.add)
                cm8 = gpool.tile([128, 8], fp32)
                ci8 = gpool.tile([128, 8], mybir.dt.uint32)
                nc.vector.max(cm8[:], Dd[:])
                nc.vector.max_index(ci8[:], cm8[:], Dd[:])
                # j* = (c2<W ? slo : shi)*W + c2 mod W, all in fp32
                c2f = gpool.tile([128, 1], fp32)
                slof = gpool.tile([128, 1], fp32)
                shif = gpool.tile([128, 1], fp32)
                ge = gpool.tile([128, 1], fp32)
                t1 = gpool.tile([128, 1], fp32)
                jf = gpool.tile([128, 1], fp32)
                nc.vector.tensor_copy(c2f[:], ci8[:, 0:1])
                nc.vector.tensor_copy(slof[:], slo[:])
                nc.vector.tensor_copy(shif[:], shi[:])
                nc.vector.tensor_scalar(
                    out=ge[:], in0=c2f[:], scalar1=float(WSUB), scalar2=None,
                    op0=mybir.AluOpType.is_ge)
                nc.vector.tensor_sub(t1[:], shif[:], slof[:])
                nc.vector.tensor_mul(t1[:], ge[:], t1[:])
                nc.vector.tensor_add(t1[:], slof[:], t1[:])  # chosen s
                nc.vector.scalar_tensor_tensor(
                    jf[:], t1[:], float(WSUB), c2f[:],
                    mybir.AluOpType.mult, mybir.AluOpType.add)
                nc.vector.scalar_tensor_tensor(
                    jf[:], ge[:], float(-WSUB), jf[:],
                    mybir.AluOpType.mult, mybir.AluOpType.add)
                ji = gpool.tile([128, 1], mybir.dt.uint32)
                nc.vector.tensor_copy(ji[:], jf[:])
                nc.gpsimd.indirect_dma_start(
                    Yg[:, rb:rb + 1], None, y_d[:],
                    IndirectOffsetOnAxis(ap=ji[:], axis=0))

            loop_cm = (tc.For_i(0, loop_n, 1) if loop_n
                       else contextlib.nullcontext())
            with loop_cm:
              pend = None  # (rb, select-state) awaiting its re-dot
              for rb in range(n_rb):
                if algo == "fold16":
                    st = emit_fold16_rb(rb)
                    if pend is not None:
                        emit_redot_rb(*pend)
                    pend = (rb, st) if st is not None else None
                    continue
                if algo in ("submax", "submax16"):
                    st = emit_submax_rb(rb)
                    if st is not None:
                        emit_redot_rb(rb, st)
                    continue
                # prefix-max of the row is built in two half-row tiles
                halves = [pms_pool.tile([128, half], fp32, name=f"pm{rb}_{h}",
                                        tag="pmh")
                          for h in range(2)]
                for t in range(n_chunk):
                    ps = psum_pool.tile([128, CHUNK], fp32)
                    emit_mms(rb, t, ps)
                    h, tc_ = divmod(t, half_chunks)
                    if parts == "mm":
                        # consume a sliver of PSUM so matmuls are not dead
                        nc.vector.tensor_copy(
                            halves[h][:, tc_ * CHUNK:tc_ * CHUNK + 8],
                            ps[:, 0:8])
                        continue
                    if t == 0:
                        initial = NEGINF
                    elif tc_ == 0:
                        initial = halves[h - 1][:, half - 1:half]
                    else:
                        initial = halves[h][:, tc_ * CHUNK - 1:tc_ * CHUNK]
                    # prefix-max of this chunk, chained to the previous chunk;
                    # data1 is an ignored operand (op1=bypass) shaped like data0.
                    nc.vector.tensor_tensor_scan(
                        halves[h][:, tc_ * CHUNK:(tc_ + 1) * CHUNK],
                        ps[:],
                        dummy[:] if mode == "bf16split" else xb[:, 0:CHUNK],
                        initial,
                        mybir.AluOpType.max,
                        mybir.AluOpType.bypass,
                    )
                if parts != "full":
                    continue
                gt = gpool.tile([128, 1], fp32)
                nc.vector.tensor_copy(gt[:], halves[1][:, half - 1:half])
                # j* = sum_j sign(g - prefix[j]) = #{j: prefix[j] < g};
                # in-place output over the prefix tiles, one accumulator per
                # half, summed later. count_engine picks ACT sign-accum or
                # DVE is_lt-accum (2x mode) per half.
                for h, Jh in ((0, J0), (1, J1)):
                    eng = {"act": "act", "dve": "dve",
                           "split": "act" if h == 0 else "dve"}[count_engine]
                    if eng == "act":
                        nc.scalar.activation(
                            halves[h][:, :],
                            halves[h][:, :],
                            mybir.ActivationFunctionType.Sign,
                            bias=gt[:],
                            scale=-1.0,
                            accum_out=Jh[:, rb:rb + 1],
                        )
                    else:
                        nc.vector.tensor_scalar(
                            out=halves[h][:, :],
                            in0=halves[h][:, :],
                            scalar1=gt[:],
                            scalar2=None,
                            op0=mybir.AluOpType.is_lt,
                            op1=mybir.AluOpType.add,
                            accum_out=Jh[:, rb:rb + 1],
                        )
                # j* for this row-block -> uint32 -> gather y[j*] from DRAM
                ji = gpool.tile([128, 1], mybir.dt.uint32, name=f"ji{rb}",
                                tag="ji")
                nc.vector.scalar_tensor_tensor(
                    ji[:], J0[:, rb:rb + 1], 1.0, J1[:, rb:rb + 1],
                    mybir.AluOpType.mult, mybir.AluOpType.add,
                )
                nc.gpsimd.indirect_dma_start(
                    Yg[:, rb:rb + 1],
                    None,
                    y_d[:],
                    IndirectOffsetOnAxis(ap=ji[:], axis=0),
                )

              if pend is not None:
                  emit_redot_rb(*pend)

            nc.sync.dma_start(out_d[:], Yg[:])

    nc.compile()
    return nc


def prep_inputs(x, xb, y, nq=NQ, m=M, mode="fp32", algo="scan"):
    """Host-side packing. Returns per-core input maps (shared arrays reused)."""
    x = np.asarray(x, dtype=np.float32)
    xb = np.asarray(xb, dtype=np.float32)
    y = np.asarray(y, dtype=np.float32)
    n_chunk = m // CHUNK
    n_rb = nq // RB
    ncores = x.shape[0] // nq
    ytab = np.ascontiguousarray(y.reshape(m, 1))
    in_maps = []

    extra = {}
    if algo in ("submax", "submax16", "fold16"):
        xaug = np.empty((K_AUG, m), np.float32)
        xaug[:DIM] = 2.0 * xb.T
        xaug[DIM] = -np.einsum("ij,ij->i", xb, xb)
        # window rows are c-major ([WSUB, K_AUG]) so the re-dot views are
        # packed (inner k stride 1) for both DVE and GPSIMD
        extra["xw"] = np.ascontiguousarray(
            xaug.reshape(K_AUG, m // WSUB, WSUB).transpose(1, 2, 0)
            .reshape(m // WSUB, K_AUG * WSUB))

    # fold16: PSUM column (chunk t, quadrant u, col w) holds augmented
    # column 32*w + 4*t + u, so the device's cross-chunk + half folds land
    # on the 32-wide sub-blocks (SM[v] = max of original block v).
    if algo == "fold16":
        tt = np.arange(n_chunk)[:, None, None]
        uu = np.arange(TPG)[None, :, None]
        ww = np.arange(JT)[None, None, :]
        col_map = (WSUB * ww + TPG * tt + uu).reshape(-1)  # [(t u w)]
    else:
        col_map = np.arange(m)

    def add_core_extras(core_maps, c):
        if algo not in ("submax", "submax16", "fold16"):
            return
        arr = np.ones((128, n_rb, K_AUG), np.float32)
        arr[:, :, :DIM] = x[c * nq:(c + 1) * nq].reshape(
            n_rb, RB, DIM).transpose(1, 0, 2)
        core_maps["xqr"] = np.ascontiguousarray(arr.reshape(128, -1))
        core_maps["xw"] = extra["xw"]

    if mode == "fp8dr":
        import ml_dtypes

        e4 = ml_dtypes.float8_e4m3fn

        def sp8(a):
            hi = a.astype(e4).astype(np.float32)
            return hi, (a - hi).astype(e4).astype(np.float32)

        A = 2.0 * xb.T                       # [16, m]
        ah, al = sp8(A)
        b2 = -np.einsum("ij,ij->i", xb, xb)  # [m]
        b2h, b2l = sp8(b2)
        # 34 rows x 2 subtiles; subtile planes share the same ref rows:
        # rows 0..15: ah, 16: b2h, 17..32: al, 33: b2l
        RB8 = np.empty((34, m), np.float32)
        RB8[0:16] = ah
        RB8[16] = b2h
        RB8[17:33] = al
        RB8[33] = b2l
        RB8 = RB8[:, col_map]                # fold16 column interleave
        xb8 = np.empty((34, 2, m), np.float32)
        xb8[:, 0] = RB8
        xb8[:, 1] = RB8
        xb8 = np.ascontiguousarray(xb8.reshape(34, 2 * m)).astype(e4)

        for c in range(ncores):
            xq = x[c * nq:(c + 1) * nq].T    # [16, nq]
            xh, xl = sp8(xq)
            Q = np.zeros((34, 2, nq), np.float32)
            Q[0:16, 0] = xh
            Q[16, 0] = 1.0
            Q[17:33, 0] = xh
            Q[33, 0] = 1.0
            Q[0:16, 1] = xl
            Q[17:33, 1] = xl
            im = {"xq8": np.ascontiguousarray(Q.reshape(34, 2 * nq)).astype(e4),
                  "xb8": xb8, "ytab": ytab}
            add_core_extras(im, c)
            in_maps.append(im)
        return in_maps

    if mode == "bf16split":
        import ml_dtypes

        bf16 = ml_dtypes.bfloat16

        def bf(a):
            return a.astype(bf16).astype(np.float32)

        a = 2.0 * xb.T                      # [16, m]
        ah, al = bf(a), a - bf(a)
        b2 = -np.einsum("ij,ij->i", xb, xb)  # [m]
        b2h, b2l = bf(b2), b2 - bf(b2)
        R = np.zeros((K_SPL, m), np.float32)
        R[0:16] = ah
        R[16] = b2h
        R[17] = b2l
        R[18:34] = ah
        R[34:50] = al
        Rr = R[:, col_map].reshape(K_SPL, n_chunk, TPG, JT)  # u = 2*v + s on axis 2
        XB2 = np.zeros((128, n_chunk * 2, JT), np.float32)
        # strip s handles u in {s, 2+s}; its column block (t*2+v) holds u=2v+s
        for s in range(2):
            XB2[64 * s:64 * s + K_SPL] = Rr[:, :, [s, 2 + s], :].transpose(
                0, 1, 2, 3).reshape(K_SPL, n_chunk * 2, JT)
        xbp = np.ascontiguousarray(
            XB2.reshape(128, n_chunk * 2 * JT)).astype(bf16)

        for c in range(ncores):
            xq = x[c * nq:(c + 1) * nq].T  # [16, nq]
            L = np.zeros((K_SPL, nq), np.float32)
            L[0:16] = bf(xq)
            L[16] = 1.0
            L[17] = 1.0
            L[18:34] = xq - bf(xq)
            L[34:50] = bf(xq)
            XQ2 = np.zeros((128, nq), np.float32)
            for s in range(2):
                XQ2[64 * s:64 * s + K_SPL] = L
            im = {"xq4": XQ2.astype(bf16), "xbp": xbp, "ytab": ytab}
            add_core_extras(im, c)
            in_maps.append(im)
        return in_maps

    # Augmented xb operand: rows 0..15 = 2*xb^T, row 16 = -||xb_j||^2.
    xaug = np.empty((K_AUG, m), np.float32)
    xaug[:DIM] = 2.0 * xb.T
    xaug[DIM] = -np.einsum("ij,ij->i", xb, xb)

    # xbp[32b+k, t*TPG+b, :] = xaug[k, col_map[t*CHUNK + b*JT : ... + JT]]
    xa = xaug[:, col_map].reshape(K_AUG, n_chunk, TPG, JT)
    xbp = np.zeros((128, n_chunk * TPG, JT), np.float32)
    for b in range(TPG):
        xbp[32 * b:32 * b + K_AUG, b::TPG, :] = xa[:, :, b, :]
    xbp = np.ascontiguousarray(xbp.reshape(128, n_chunk * TPG * JT))

    for c in range(ncores):
        xq = x[c * nq:(c + 1) * nq]  # [nq, 16]
        xq4 = np.zeros((128, nq), np.float32)
        for b in range(TPG):
            xq4[32 * b:32 * b + DIM] = xq.T
            xq4[32 * b + DIM] = 1.0
        im = {"xq4": xq4, "xbp": xbp, "ytab": ytab}
        add_core_extras(im, c)
        in_maps.append(im)
    return in_maps


def unpack_output(out_np, nq=NQ):
    """[128, n_rb] device layout -> [nq] query order."""
    return np.ascontiguousarray(out_np.T).reshape(nq)


_NC_CACHE = {}
MODE = "bf16split"
ALGO = "fold16"


def kernel(x, xb, y):
    import concourse.bass_utils as bass_utils

    key = (MODE, ALGO)
    if key not in _NC_CACHE:
        _NC_CACHE[key] = build_nc(mode=MODE, algo=ALGO)
    nc = _NC_CACHE[key]

    in_maps = prep_inputs(x, xb, y, mode=MODE, algo=ALGO)
    res = bass_utils.run_bass_kernel_spmd(nc, in_maps, core_ids=list(range(NCORES)))
    outs = [unpack_output(r["yout"]) for r in res.results]
    return np.concatenate(outs).astype(np.float32)


if __name__ == "__main__":
    # smoke test with random data against numpy reference
    rng = np.random.default_rng(0)
    x = rng.standard_normal((N, DIM), dtype=np.float32)
    xb = rng.standard_normal((M, DIM), dtype=np.float32)
    y = rng.random(M, dtype=np.float32)
    got = kernel(x, xb, y)
    d2 = (np.sum(x * x, 1)[:, None] + np.sum(xb * xb, 1)[None, :]
          - 2.0 * x @ xb.T)
    want = y[np.argmin(d2, axis=1)]
    err = np.abs(got - want)
    print("mismatches:", int((err > 0).sum()), "/", N)

